# revision 15
# baseline (speedup 1.0000x reference)
"""Trainium2 Bass kernel for nn_CausalCrisisModel (data-parallel over 8 cores).

Per-core batch N=512, activations feature-major in SBUF [feature, batch].
Linears: out.T = W @ X.T on PE (lhsT = W.T tile, rhs = X.T tile), fp32
throughout (a near-tie in the hardest-centroid argmax needs ~1e-6 element
accuracy along the c path). Gelu via Erf LUT (4 ULP, 0.5 folded into the next
layer's weights), LN rsqrt via Sqrt LUT + one Newton step, 2-token softmax via
sigmoid of the score difference with K-diff computed before the projection,
centroid argmax via max+is_equal one-hot matmul (|c|^2 term dropped).
"""
import os
import numpy as np

B, D, H, HD, ND, MIX = 4096, 1024, 4, 256, 7, 0.3
NCORES = 8
N = B // NCORES          # 512
BN = D // 4              # 256
EPS = 1e-5
ISQ2 = float(1.0 / np.sqrt(2.0))
_CACHE = {}
LAST_EXEC_NS = None

LINS = {
    "saI_l1": (D, BN), "saI_l2": (BN, D),
    "saT_l1": (D, BN), "saT_l2": (BN, D),
    "g_cI": (D, D), "g_cT": (D, D),
    "q1": (D, D), "q2": (D, D), "k1": (D, D), "k2": (D, D),
    "v": (D, D), "o": (D, D),
    "lam1": (2 * D, BN), "lam2": (BN, 1),
    "c1": (2 * D, 2 * D), "c2": (2 * D, D),
    "s1": (2 * D, 2 * D), "s2": (2 * D, D),
    "d1": (2 * D, 2 * D), "d2": (2 * D, 2 * D),
    "m1": (D, 128), "m2": (128, 64), "m3": (64, ND),
}
LNS = {"saI": BN, "saT": BN, "lnI": D, "lnT": D,
       "cln1": 2 * D, "cln2": D, "sln1": 2 * D, "sln2": D}
GELU_LNS = ("saI", "saT", "cln1", "sln1")


def _f32(x):
    return np.ascontiguousarray(np.asarray(x), dtype=np.float32)


def _param_layout():
    """Deterministic packing of all [*,1] params into one [128, C] tensor."""
    cols = {}
    order = []
    for t, (di, do) in LINS.items():
        for m in range((do + 127) // 128):
            order.append((f"b_{t}", m, min(128, do - m * 128)))
    for m in range((2 * D) // 128):
        order.append(("b_d1s", m, 128))
    for t, d in LNS.items():
        for k in range(d // 128):
            order.append((f"g_{t}", k, 128))
            order.append((f"be_{t}", k, 128))
            if t in GELU_LNS:
                order.append((f"ge_{t}", k, 128))
                order.append((f"bee_{t}", k, 128))
    for j, (name, k, rows) in enumerate(order):
        cols[(name, k)] = (j, rows)
    return cols, len(order)


def _build(cmul):
    import concourse.mybir as mybir
    import concourse.tile as tile
    from concourse import bacc

    f32 = mybir.dt.float32
    AF = mybir.ActivationFunctionType
    OP = mybir.AluOpType
    AX = mybir.AxisListType

    nc = bacc.Bacc("TRN2", target_bir_lowering=False, debug=False)

    PCOLS, NPC = _param_layout()
    dr = {}
    for t, (di, do) in LINS.items():
        dr[f"wT_{t}"] = nc.dram_tensor(f"wT_{t}", [di, do], f32, kind="ExternalInput")
    dr["pmega"] = nc.dram_tensor("pmega", [128, NPC], f32, kind="ExternalInput")
    dr["fIT"] = nc.dram_tensor("fIT", [D, N], f32, kind="ExternalInput")
    dr["fTT"] = nc.dram_tensor("fTT", [D, N], f32, kind="ExternalInput")
    dr["cm2T"] = nc.dram_tensor("cm2T", [D, ND], f32, kind="ExternalInput")
    dr["centmix"] = nc.dram_tensor("centmix", [ND, D], f32, kind="ExternalInput")
    dr["mu2"] = nc.dram_tensor("mu2", [1, ND], f32, kind="ExternalInput")
    dr["selcol"] = nc.dram_tensor("selcol", [128, 16], f32, kind="ExternalInput")
    dr["selrow"] = nc.dram_tensor("selrow", [4, 512], f32, kind="ExternalInput")
    outT = nc.dram_tensor("outT", [4 * D + ND, N], f32, kind="ExternalOutput")

    with tile.TileContext(nc) as tc:
        from contextlib import ExitStack
        ctx = ExitStack()
        acts = ctx.enter_context(tc.tile_pool(name="acts", bufs=1))
        wpool = ctx.enter_context(tc.tile_pool(name="w", bufs=1))
        ppool = ctx.enter_context(tc.tile_pool(name="par", bufs=1))
        rows = ctx.enter_context(tc.tile_pool(name="rows", bufs=1))
        mmps = ctx.enter_context(tc.tile_pool(name="mmps", bufs=3, space="PSUM"))
        bcps = ctx.enter_context(tc.tile_pool(name="bcps", bufs=3, space="PSUM"))
        smps = ctx.enter_context(tc.tile_pool(name="smps", bufs=2, space="PSUM"))

        NSLOT = 64
        free_slots = list(range(NSLOT))
        _nm = [0]

        def nname(pfx):
            _nm[0] += 1
            return f"{pfx}{_nm[0]}"

        class Tl:
            __slots__ = ("slot", "t")

            def __init__(self):
                self.slot = free_slots.pop()
                self.t = acts.tile([128, N], f32, tag=f"a{self.slot}", name=nname("act"))

            def ap(self):
                return self.t[:]

            def free(self):
                if self.slot is not None:
                    free_slots.append(self.slot)
                    self.slot = None

        def anew():
            return Tl()

        def afree(ts):
            for t in ts:
                t.free()

        pmt = ppool.tile([128, NPC], f32, tag="pmt", bufs=1)
        nc.sync.dma_start(pmt[:], dr["pmega"].ap()[:, :])

        def ptile(name, k, rows_n=None):
            j, rows = PCOLS[(name, k)]
            return pmt[:rows, j:j + 1]

        def rtile(p, tag="r", bufs=5):
            return rows.tile([p, N], f32, tag=tag, bufs=bufs, name=nname("row"))

        ones_col = ppool.tile([128, 1], f32, tag="ones_col", bufs=1)
        nc.vector.memset(ones_col[:], 1.0)
        ones_r128 = rows.tile([1, 128], f32, tag="ones_r128", bufs=1)
        nc.vector.memset(ones_r128[:], 1.0)
        ones_row = rows.tile([1, N], f32, tag="ones_row", bufs=1)
        nc.vector.memset(ones_row[:], 1.0)
        ones14 = rows.tile([1, 4], f32, tag="ones14", bufs=1)
        nc.vector.memset(ones14[:], 1.0)
        selcol_t = ppool.tile([128, 16], f32, tag="selcol", bufs=1)
        nc.sync.dma_start(selcol_t[:], dr["selcol"].ap()[:, :])
        selrow_t = rows.tile([4, 512], f32, tag="selrow", bufs=1)
        nc.sync.dma_start(selrow_t[:], dr["selrow"].ap()[:, :])

        def linear(name, src, evict, chunk=8):
            """src: list of Tl k-tiles; evict(m, psum) consumes each psum."""
            di, do = LINS[name]
            kt, mt = di // 128, do // 128
            assert len(src) == kt and do % 128 == 0
            for m in range(mt):
                ps = mmps.tile([128, N], f32, tag="mm", name=nname("mmp"))
                k = 0
                for k0 in range(0, kt, chunk):
                    kb = min(chunk, kt - k0)
                    wt = wpool.tile([128, kb * 128], f32, tag="wsml", bufs=4, name=nname("w"))
                    src_ap = dr[f"wT_{name}"].ap().rearrange(
                        "(kt p) n -> p kt n", p=128)
                    nc.sync.dma_start(
                        wt[:].rearrange("p (kt n) -> p kt n", n=128),
                        src_ap[:, k0:k0 + kb, m * 128:(m + 1) * 128])
                    for kk in range(kb):
                        nc.tensor.matmul(ps[:], wt[:, kk * 128:(kk + 1) * 128],
                                         src[k].ap(), start=(k == 0),
                                         stop=(k == kt - 1))
                        k += 1
                evict(m, ps)

        def evict_bias(name, out_list, func=None):
            AFunc = func or AF.Identity

            def ev(m, ps):
                t = anew()
                nc.scalar.activation(t.ap(), ps[:], AFunc,
                                     bias=ptile(f"b_{name}", m))
                out_list.append(t)
            return ev

        def ln_stats(tag, x):
            """Phase 1: PE stats matmuls + DVE row math. Returns state."""
            d = LNS[tag]
            kt = d // 128
            assert len(x) == kt
            st = smps.tile([33, N], f32, tag="stats", name=nname("st"))
            stA = st[0:1, :]    # x-sum, PE col group 0
            stB = st[32:33, :]  # x^2-sum, PE col group 1 (concurrent)
            for k in range(kt):
                sq = rtile(128, tag="sqr", bufs=2)
                nc.scalar.activation(sq[:], x[k].ap(), AF.Square)
                nc.tensor.matmul(stA, ones_col[:], x[k].ap(),
                                 start=(k == 0), stop=(k == kt - 1),
                                 skip_group_check=True)
                nc.tensor.matmul(stB, ones_col[:], sq[:],
                                 start=(k == 0), stop=(k == kt - 1),
                                 skip_group_check=True)
            mu = rtile(1)
            nc.vector.tensor_scalar(mu[:], stA, 1.0 / d, None, OP.mult)
            q = rtile(1)
            nc.vector.tensor_scalar(q[:], stB, 1.0 / d, None, OP.mult)
            u = rtile(1)
            nc.vector.scalar_tensor_tensor(u[:], mu[:], -1.0, mu[:], OP.mult, OP.mult)
            nc.vector.tensor_add(u[:], u[:], q[:])
            nc.vector.tensor_scalar(u[:], u[:], EPS, None, OP.add)
            r = rtile(1)
            nc.scalar.activation(r[:], u[:], AF.Sqrt)
            rc = rtile(1)
            nc.vector.reciprocal(rc[:], u[:])
            nc.vector.tensor_mul(r[:], r[:], rc[:])
            nc.vector.tensor_mul(rc[:], r[:], r[:])
            nc.vector.tensor_mul(rc[:], rc[:], u[:])
            nc.vector.tensor_scalar(rc[:], rc[:], -0.5, 1.5, OP.mult, OP.add)
            nc.vector.tensor_mul(r[:], r[:], rc[:])
            nc.vector.scalar_tensor_tensor(mu[:], mu[:], -1.0, r[:],
                                           OP.mult, OP.mult)
            return (tag, x, r, mu)

        def ln_apply(state, gelu=False):
            """Phase 2: broadcast matmuls + per-tile normalize (+erf-gelu)."""
            tag, x, r, mu = state
            kt = LNS[tag] // 128
            bs = bcps.tile([128, N], f32, tag="bc", name=nname("bc"))
            nc.tensor.matmul(bs[:], ones_r128[:], r[:], start=True, stop=True)
            bt = bcps.tile([128, N], f32, tag="bc", name=nname("bc"))
            nc.tensor.matmul(bt[:], ones_r128[:], mu[:], start=True, stop=True)
            sb = rtile(128, tag="bcs", bufs=2)
            nc.vector.tensor_copy(sb[:], bs[:])
            tb = rtile(128, tag="bcs", bufs=2)
            nc.vector.tensor_copy(tb[:], bt[:])
            out = []
            for k in range(kt):
                nrm = anew()
                nc.vector.tensor_mul(nrm.ap(), x[k].ap(), sb[:])
                nc.vector.tensor_add(nrm.ap(), nrm.ap(), tb[:])
                if not gelu:
                    t = anew()
                    nc.scalar.activation(t.ap(), nrm.ap(), AF.Identity,
                                         bias=ptile(f"be_{tag}", k),
                                         scale=ptile(f"g_{tag}", k))
                    out.append(t)
                    nrm.free()
                else:
                    xln = anew()
                    nc.scalar.activation(xln.ap(), nrm.ap(), AF.Identity,
                                         bias=ptile(f"be_{tag}", k),
                                         scale=ptile(f"g_{tag}", k))
                    er = anew()
                    nc.scalar.activation(er.ap(), nrm.ap(), AF.Erf,
                                         bias=ptile(f"bee_{tag}", k),
                                         scale=ptile(f"ge_{tag}", k))
                    t = anew()
                    nc.vector.scalar_tensor_tensor(t.ap(), er.ap(), 1.0, xln.ap(),
                                                   OP.add, OP.mult)
                    out.append(t)
                    xln.free(); er.free(); nrm.free()
            return out

        def layer_norm(tag, x, gelu=False):
            return ln_apply(ln_stats(tag, x), gelu=gelu)

        # ================= forward =================
        fI, fT = [], []
        for k in range(8):
            t = anew(); nc.gpsimd.dma_start(t.ap(), dr["fIT"].ap()[k * 128:(k + 1) * 128, :]); fI.append(t)
            t = anew(); nc.scalar.dma_start(t.ap(), dr["fTT"].ap()[k * 128:(k + 1) * 128, :]); fT.append(t)

        def sa_l2(br, fin, hg):
            out = []

            def ev(m, ps):
                t = anew()
                nc.scalar.activation(t.ap(), ps[:], AF.Identity,
                                     bias=ptile(f"b_sa{br}_l2", m))
                nc.vector.tensor_add(t.ap(), t.ap(), fin[m].ap())
                out.append(t)
            linear(f"sa{br}_l2", hg, ev)
            return out

        # interleaved: saT l1 matmuls cover saI LN row math, etc.
        h1I = []
        linear("saI_l1", fI, evict_bias("saI_l1", h1I), chunk=2)
        stI = ln_stats("saI", h1I)
        h1T = []
        linear("saT_l1", fT, evict_bias("saT_l1", h1T))
        stT = ln_stats("saT", h1T)
        hgI = ln_apply(stI, gelu=True)
        afree(h1I)
        fIr = sa_l2("I", fI, hgI)
        afree(hgI)
        hgT = ln_apply(stT, gelu=True)
        afree(h1T)
        fTr = sa_l2("T", fT, hgT)
        afree(hgT)
        afree(fI); afree(fT)

        def gca_pre(cname, kv, res):
            # o_proj(v_proj(kv)) composed into one matmul on the host
            pre = []

            def ev(m, ps):
                t = anew()
                nc.scalar.activation(t.ap(), ps[:], AF.Identity,
                                     bias=ptile(f"b_{cname}", m))
                nc.vector.tensor_add(t.ap(), t.ap(), res[m].ap())
                pre.append(t)
            linear(cname, kv, ev)
            return pre

        preI = gca_pre("g_cI", fTr, fIr)
        stLI = ln_stats("lnI", preI)
        preT = gca_pre("g_cT", fIr, fTr)
        stLT = ln_stats("lnT", preT)
        afree(fIr); afree(fTr)
        oI = ln_apply(stLI)
        afree(preI)
        # token-0 q1 projection (only needs oI) covers lnT row math
        q1_t0, v0 = [], []
        linear("q1", oI, evict_bias("q1", q1_t0))
        oT = ln_apply(stLT)
        afree(preT)
        fused = oI + oT  # 16 k-tiles

        # lambda path (small) — interleaved into attention below
        lam_h = []
        linear("lam1", fused, evict_bias("lam1", lam_h, func=AF.Relu))
        wl2 = wpool.tile([128, 2], f32, tag="wtiny", bufs=2, name=nname("w"))
        nc.sync.dma_start(
            wl2[:].rearrange("p (kt n) -> p kt n", n=1),
            dr["wT_lam2"].ap().rearrange("(kt p) n -> p kt n", p=128)[:, :, :])
        lam_ps = smps.tile([1, N], f32, tag="stats", name=nname("st"))
        for k in range(2):
            nc.tensor.matmul(lam_ps[:], wl2[:, k:k + 1], lam_h[k].ap(),
                             start=(k == 0), stop=(k == 1))
        afree(lam_h)
        lam = rtile(1, tag="lam", bufs=1)
        nc.scalar.activation(lam[:], lam_ps[:], AF.Sigmoid,
                             bias=ptile("b_lam2", 0, 1))
        lam4_ps = smps.tile([4, N], f32, tag="stats", name=nname("st"))
        nc.tensor.matmul(lam4_ps[:], ones14[:], lam[:], start=True, stop=True)
        lam4 = rtile(4, tag="lam4", bufs=1)
        nc.vector.tensor_copy(lam4[:], lam4_ps[:])
        om4 = rtile(4, tag="om4", bufs=1)
        nc.vector.tensor_scalar(om4[:], lam4[:], -1.0, 1.0, OP.mult, OP.add)

        # K-diff projections: K0-K1 = W_k @ (tok0 - tok1)
        tdf = []
        for k in range(8):
            t = anew()
            nc.vector.tensor_sub(t.ap(), oI[k].ap(), oT[k].ap())
            tdf.append(t)

        def ev_copy(lst):
            def ev(m, ps):
                t = anew()
                nc.scalar.copy(t.ap(), ps[:])
                lst.append(t)
            return ev
        kd1, kd2 = [], []
        linear("k1", tdf, ev_copy(kd1))
        linear("k2", tdf, ev_copy(kd2))
        afree(tdf)

        SCALE = float(HD ** -0.5)

        def score_rows2(q1t, q2t):
            """Both branch scores packed into PE col groups 0 and 1."""
            sc = smps.tile([36, N], f32, tag="stats", name=nname("st"))
            for k in range(8):
                p1 = rtile(128, tag="prod", bufs=4)
                nc.vector.tensor_mul(p1[:], q1t[k].ap(), kd1[k].ap())
                nc.tensor.matmul(sc[0:4, :], selcol_t[:, 4 * (k // 2):4 * (k // 2) + 4],
                                 p1[:], start=(k == 0), stop=(k == 7),
                                 skip_group_check=True)
                p2 = rtile(128, tag="prod", bufs=4)
                nc.vector.tensor_mul(p2[:], q2t[k].ap(), kd2[k].ap())
                nc.tensor.matmul(sc[32:36, :], selcol_t[:, 4 * (k // 2):4 * (k // 2) + 4],
                                 p2[:], start=(k == 0), stop=(k == 7),
                                 skip_group_check=True)
            a1 = rtile(4, tag="arow", bufs=3)
            nc.scalar.activation(a1[:], sc[0:4, :], AF.Sigmoid, scale=SCALE)
            a2 = rtile(4, tag="arow", bufs=3)
            nc.scalar.activation(a2[:], sc[32:36, :], AF.Sigmoid, scale=SCALE)
            return a1, a2

        def attn_w(a1, a2, ti):
            u = rtile(4, tag="arow", bufs=3)
            nc.vector.tensor_mul(u[:], lam4[:], a2[:])
            nc.vector.tensor_sub(u[:], a1[:], u[:])
            w0 = rtile(4, tag=f"w{ti}0", bufs=1)
            nc.vector.tensor_scalar(w0[:], u[:], 0.0, None, OP.max)
            w1 = rtile(4, tag=f"w{ti}1", bufs=1)
            nc.vector.tensor_sub(w1[:], om4[:], u[:])
            nc.vector.tensor_scalar(w1[:], w1[:], 0.0, None, OP.max)
            return w0, w1

        # token 0 (q1_t0 computed above, during lnT apply window)
        q2_t0 = []
        linear("q2", oI, evict_bias("q2", q2_t0))
        a1_t0, a2_t0 = score_rows2(q1_t0, q2_t0)
        afree(q1_t0); afree(q2_t0)
        wq0 = attn_w(a1_t0, a2_t0, 0)
        linear("v", oI, evict_bias("v", v0))
        # token 1
        q1_t1 = []
        linear("q1", oT, evict_bias("q1", q1_t1))
        q2_t1 = []
        linear("q2", oT, evict_bias("q2", q2_t1))
        a1_t1, a2_t1 = score_rows2(q1_t1, q2_t1)
        afree(q1_t1); afree(q2_t1)
        wq1 = attn_w(a1_t1, a2_t1, 1)
        afree(kd1); afree(kd2)
        v1 = []
        linear("v", oT, evict_bias("v", v1))

        x = []

        def combine_oproj(ti, w01, res):
            w0, w1 = w01
            att = []
            for h in range(4):
                b0 = bcps.tile([128, N], f32, tag="bc", name=nname("bc"))
                nc.tensor.matmul(b0[:], selrow_t[:, 128 * h:128 * (h + 1)], w0[:],
                                 start=True, stop=True)
                b1 = bcps.tile([128, N], f32, tag="bc", name=nname("bc"))
                nc.tensor.matmul(b1[:], selrow_t[:, 128 * h:128 * (h + 1)], w1[:],
                                 start=True, stop=True)
                for k in (2 * h, 2 * h + 1):
                    r1 = rtile(128, tag="prod", bufs=4)
                    nc.vector.tensor_mul(r1[:], v0[k].ap(), b0[:])
                    t = anew()
                    nc.vector.tensor_mul(t.ap(), v1[k].ap(), b1[:])
                    nc.vector.tensor_add(t.ap(), t.ap(), r1[:])
                    att.append(t)

            def ev(m, ps, _res=res):
                t = anew()
                nc.scalar.activation(t.ap(), ps[:], AF.Identity,
                                     bias=ptile("b_o", m))
                nc.vector.tensor_add(t.ap(), t.ap(), _res[m].ap())
                x.append(t)
            linear("o", att, ev)
            afree(att)

        combine_oproj(0, wq0, oI)
        afree(oI)
        combine_oproj(1, wq1, oT)
        afree(v0); afree(v1)
        afree(oT)

        # ---- c/s paths interleaved, cdist launched early ----
        h1c = []
        linear("c1", x, evict_bias("c1", h1c))
        st_c1 = ln_stats("cln1", h1c)
        h1s = []
        linear("s1", x, evict_bias("s1", h1s))
        st_s1 = ln_stats("sln1", h1s)
        afree(x)
        hgc = ln_apply(st_c1, gelu=True)
        afree(h1c)
        h2c = []
        linear("c2", hgc, evict_bias("c2", h2c))
        afree(hgc)
        st_c2 = ln_stats("cln2", h2c)
        hgs = ln_apply(st_s1, gelu=True)
        afree(h1s)
        h2s = []
        linear("s2", hgs, evict_bias("s2", h2s))
        afree(hgs)
        st_s2 = ln_stats("sln2", h2s)
        c = ln_apply(st_c2)
        afree(h2c)

        # cdist part 1: dt2 = ||mu||^2 - 2 mu.c -> one-hot (gpsimd latency
        # covered by the s apply + d1/d2 matmuls below)
        d2ps = smps.tile([8, N], f32, tag="stats", name=nname("st"))
        wcm = wpool.tile([128, 8 * ND], f32, tag="wtiny", bufs=2, name=nname("w"))
        nc.sync.dma_start(
            wcm[:].rearrange("p (kt n) -> p kt n", n=ND),
            dr["cm2T"].ap().rearrange("(kt p) n -> p kt n", p=128)[:, :, :])
        for k in range(8):
            nc.tensor.matmul(d2ps[:ND, :], wcm[:, k * ND:(k + 1) * ND], c[k].ap(),
                             start=(k == 0), stop=False)
        mu2t = rows.tile([1, ND], f32, tag="mu2", bufs=1, name=nname("row"))
        nc.sync.dma_start(mu2t[:], dr["mu2"].ap()[:, :])
        nc.tensor.matmul(d2ps[:ND, :], mu2t[:], ones_row[:], start=False, stop=True)
        d2s = rtile(ND, tag="d2s", bufs=1)
        nc.vector.tensor_copy(d2s[:], d2ps[:ND, :])
        mx7 = rtile(ND, tag="mx7", bufs=1)
        import concourse.bass_isa as bass_isa
        nc.gpsimd.partition_all_reduce(mx7[:], d2s[:], channels=ND,
                                       reduce_op=bass_isa.ReduceOp.max)
        oneh = rtile(ND, tag="oneh", bufs=1)
        nc.vector.tensor_tensor(oneh[:], d2s[:], mx7[:], OP.is_equal)

        s = ln_apply(st_s2)
        afree(h2s)

        # domain classifier stage 1 on s (m2/m3 issued later, between
        # dense d1/hardest streams, so PE never idles on the thin chain)
        dm1 = []
        linear("m1", s, evict_bias("m1", dm1, func=AF.Relu))
        wm2 = wpool.tile([128, 64], f32, tag="wtiny", bufs=2, name=nname("w"))
        nc.sync.dma_start(wm2[:], dr["wT_m2"].ap()[:, :])
        wm3 = wpool.tile([128, ND], f32, tag="wtiny", bufs=2, name=nname("w"))
        nc.sync.dma_start(wm3[:64, :], dr["wT_m3"].ap()[:, :])

        # h_recon = d2(gelu(d1([c, s]))), 0.5 folded into wT_d2
        cat = c + s
        hg = []

        def ev_d1(m, ps):
            xt = anew()
            nc.scalar.activation(xt.ap(), ps[:], AF.Identity,
                                 bias=ptile("b_d1", m))
            er = anew()
            nc.scalar.activation(er.ap(), ps[:], AF.Erf,
                                 bias=ptile("b_d1s", m), scale=ISQ2)
            t = anew()
            nc.vector.scalar_tensor_tensor(t.ap(), er.ap(), 1.0, xt.ap(),
                                           OP.add, OP.mult)
            hg.append(t)
            xt.free(); er.free()
        linear("d1", cat, ev_d1)

        ps2 = mmps.tile([128, N], f32, tag="mm", name=nname("mmp"))
        nc.tensor.matmul(ps2[:64, :], wm2[:, :64], dm1[0].ap(),
                         start=True, stop=True)
        afree(dm1)
        dm2 = anew()
        nc.scalar.activation(dm2.ap()[:64, :], ps2[:64, :], AF.Relu,
                             bias=ptile("b_m2", 0))

        # c_do = cmul*c + centmix.T @ onehot (before d2 so c frees early
        # and the thin hardest matmuls hide inside the dense stream)
        cmixt = wpool.tile([ND, D], f32, tag="cmix", bufs=1, name=nname("w"))
        nc.sync.dma_start(cmixt[:], dr["centmix"].ap()[:, :])
        for m in range(8):
            hp = mmps.tile([128, N], f32, tag="mm", name=nname("mmp"))
            nc.tensor.matmul(hp[:], cmixt[:, m * 128:(m + 1) * 128], oneh[:],
                             start=True, stop=True)
            cd = anew()
            nc.vector.scalar_tensor_tensor(cd.ap(), c[m].ap(), cmul, hp[:],
                                           OP.mult, OP.add)
            nc.sync.dma_start(outT.ap()[m * 128:(m + 1) * 128, :], cd.ap())
            cd.free()
        afree(c)

        ps3 = mmps.tile([128, N], f32, tag="mm", name=nname("mmp"))
        nc.tensor.matmul(ps3[:ND, :], wm3[:64, :ND], dm2.ap()[:64, :],
                         start=True, stop=True)
        dm2.free()
        dm3 = anew()
        nc.scalar.activation(dm3.ap()[:ND, :], ps3[:ND, :], AF.Identity,
                             bias=ptile("b_m3", 0))
        nc.sync.dma_start(outT.ap()[4 * D:4 * D + ND, :], dm3.ap()[:ND, :])
        dm3.free()

        def ev_d2(m, ps):
            t = anew()
            nc.scalar.activation(t.ap(), ps[:], AF.Identity,
                                 bias=ptile("b_d2", m))
            nc.sync.dma_start(
                outT.ap()[2 * D + m * 128:2 * D + (m + 1) * 128, :], t.ap())
            t.free()
        linear("d2", hg, ev_d2)
        afree(hg)

        for m, t in enumerate(s):
            nc.sync.dma_start(outT.ap()[D + m * 128:D + (m + 1) * 128, :], t.ap())
            t.free()

        ctx.close()

    nc.compile()
    return nc


def _prepare_inputs(f_I, f_T, params, centroids, counts):
    p = params
    W = {}

    def put(tag, lin, half=False):
        w = _f32(lin["w"])
        b = _f32(lin["b"])
        wT = np.ascontiguousarray(w.T)
        if half:
            wT = np.ascontiguousarray(np.float32(0.5) * wT)
        W[f"wT_{tag}"] = wT
        W[f"b_{tag}"] = np.ascontiguousarray(b.reshape(-1, 1))

    put("saI_l1", p["sa_I"]["l1"]); put("saI_l2", p["sa_I"]["l2"], half=True)
    put("saT_l1", p["sa_T"]["l1"]); put("saT_l2", p["sa_T"]["l2"], half=True)
    for br in ("I", "T"):
        wv = _f32(p["gca"][f"v_{br}"]["w"]).astype(np.float64)
        bv = _f32(p["gca"][f"v_{br}"]["b"]).astype(np.float64)
        wo = _f32(p["gca"][f"o_{br}"]["w"]).astype(np.float64)
        bo = _f32(p["gca"][f"o_{br}"]["b"]).astype(np.float64)
        W[f"wT_g_c{br}"] = np.ascontiguousarray((wo @ wv).astype(np.float32).T)
        W[f"b_g_c{br}"] = np.ascontiguousarray(
            (wo @ bv + bo).astype(np.float32).reshape(-1, 1))
    for t in ("q1", "q2", "k1", "k2", "v", "o", "lam1", "lam2"):
        put(t, p["diff"][t])
    put("c1", p["dis"]["c1"]); put("c2", p["dis"]["c2"], half=True)
    put("s1", p["dis"]["s1"]); put("s2", p["dis"]["s2"], half=True)
    put("d1", p["dis"]["d1"]); put("d2", p["dis"]["d2"], half=True)
    put("m1", p["dom"]["l1"]); put("m2", p["dom"]["l2"]); put("m3", p["dom"]["l3"])

    LNMAP = {"saI": p["sa_I"]["ln"], "saT": p["sa_T"]["ln"],
             "lnI": p["gca"]["ln_I"], "lnT": p["gca"]["ln_T"],
             "cln1": p["dis"]["cln1"], "cln2": p["dis"]["cln2"],
             "sln1": p["dis"]["sln1"], "sln2": p["dis"]["sln2"]}
    ISQ = np.float32(ISQ2)
    for t, ln in LNMAP.items():
        g = _f32(ln["g"]).reshape(-1, 1)
        b = _f32(ln["b"]).reshape(-1, 1)
        W[f"g_{t}"] = np.ascontiguousarray(g)
        W[f"be_{t}"] = np.ascontiguousarray(b)
        if t in GELU_LNS:
            W[f"ge_{t}"] = np.ascontiguousarray(g * ISQ)
            W[f"bee_{t}"] = np.ascontiguousarray(b * ISQ)
    W["b_d1s"] = np.ascontiguousarray(W["b_d1"] * ISQ)

    cent = _f32(centroids)
    counts_sum = float(np.asarray(counts, np.float64).sum())
    fmix = MIX if counts_sum > 0 else 0.0
    W["cm2T"] = np.ascontiguousarray((np.float32(-2.0) * cent).T)
    W["centmix"] = np.ascontiguousarray(np.float32(fmix) * cent)
    try:
        import jax
        cpu = jax.devices("cpu")[0]
        with jax.default_device(cpu):
            import jax.numpy as jnp
            mu2 = np.asarray((jnp.asarray(cent) ** 2).sum(-1), np.float32)
    except Exception:
        mu2 = (cent.astype(np.float64) ** 2).sum(-1).astype(np.float32)
    W["mu2"] = np.ascontiguousarray(mu2.reshape(1, ND))

    selcol = np.zeros((128, 16), np.float32)
    selrow = np.zeros((4, 512), np.float32)
    for h in range(4):
        selcol[:, 4 * h + h] = 1.0
        selrow[h, 128 * h:128 * (h + 1)] = 1.0
    W["selcol"] = selcol
    W["selrow"] = selrow

    PCOLS, NPC = _param_layout()
    pmega = np.zeros((128, NPC), np.float32)
    for (name, k), (j, rows) in PCOLS.items():
        pmega[:rows, j] = W[name][k * 128:k * 128 + rows, 0]
    W["pmega"] = pmega

    fI = _f32(f_I)
    fT = _f32(f_T)
    in_maps = []
    for cix in range(NCORES):
        m = dict(W)
        m["fIT"] = np.ascontiguousarray(fI[cix * N:(cix + 1) * N, :].T)
        m["fTT"] = np.ascontiguousarray(fT[cix * N:(cix + 1) * N, :].T)
        in_maps.append(m)
    return in_maps, float(1.0 - fmix)


def kernel(f_I, f_T, params, centroids, counts):
    from concourse import bass_utils

    in_maps, cmul = _prepare_inputs(f_I, f_T, params, centroids, counts)
    key = ("prog", cmul)
    if key not in _CACHE:
        _CACHE[key] = _build(cmul)
    nc = _CACHE[key]

    trace = os.environ.get("KERNEL_TRACE") == "1"
    if trace:
        try:
            import sys, types
            if "antenv.axon_hooks" not in sys.modules:
                import antenv  # noqa: F401
                from trn_agent_boot.trn_boot import _ntff_profile_via_ctypes
                hook = _ntff_profile_via_ctypes("/opt/axon/libaxon_pjrt.so")
                mod = types.ModuleType("antenv.axon_hooks")
                mod.get_axon_ntff_profile_hook = lambda: hook
                mod.set_axon_ntff_profile_hook = lambda h: None
                sys.modules["antenv.axon_hooks"] = mod
        except Exception as e:
            print("trace hook install failed:", e)
            trace = False

    res = bass_utils.run_bass_kernel_spmd(
        nc, in_maps, core_ids=list(range(NCORES)), trace=trace)
    global LAST_EXEC_NS
    if trace and res.exec_time_ns is not None:
        LAST_EXEC_NS = int(res.exec_time_ns)
        print(f"HW exec time: {res.exec_time_ns} ns")

    out = np.empty((B, 4 * D + ND), np.float32)
    for cix in range(NCORES):
        out[cix * N:(cix + 1) * N, :] = res.results[cix]["outT"].T
    return out


# revision 16
# speedup vs baseline: 1.0281x; 1.0281x over previous
"""Trainium2 Bass kernel for nn_CausalCrisisModel (data-parallel over 8 cores).

Per-core batch N=512, activations feature-major in SBUF [feature, batch].
Linears: out.T = W @ X.T on PE (lhsT = W.T tile, rhs = X.T tile), fp32
throughout (a near-tie in the hardest-centroid argmax needs ~1e-6 element
accuracy along the c path). Gelu via Erf LUT (4 ULP, 0.5 folded into the next
layer's weights), LN rsqrt via Sqrt LUT + one Newton step, 2-token softmax via
sigmoid of the score difference with K-diff computed before the projection,
centroid argmax via max+is_equal one-hot matmul (|c|^2 term dropped).
"""
import os
import numpy as np

B, D, H, HD, ND, MIX = 4096, 1024, 4, 256, 7, 0.3
NCORES = 8
N = B // NCORES          # 512
BN = D // 4              # 256
EPS = 1e-5
ISQ2 = float(1.0 / np.sqrt(2.0))
_CACHE = {}
LAST_EXEC_NS = None

LINS = {
    "saI_l1": (D, BN), "saI_l2": (BN, D),
    "saT_l1": (D, BN), "saT_l2": (BN, D),
    "g_cI": (D, D), "g_cT": (D, D),
    "q1": (D, D), "q2": (D, D), "k1": (D, D), "k2": (D, D),
    "v": (D, D), "o": (D, D),
    "lam1": (2 * D, BN), "lam2": (BN, 1),
    "c1": (2 * D, 2 * D), "c2": (2 * D, D),
    "s1": (2 * D, 2 * D), "s2": (2 * D, D),
    "d1": (2 * D, 2 * D), "d2": (2 * D, 2 * D),
    "m1": (D, 128), "m2": (128, 64), "m3": (64, ND),
}
LNS = {"saI": BN, "saT": BN, "lnI": D, "lnT": D,
       "cln1": 2 * D, "cln2": D, "sln1": 2 * D, "sln2": D}
GELU_LNS = ("saI", "saT", "cln1", "sln1")


def _f32(x):
    return np.ascontiguousarray(np.asarray(x), dtype=np.float32)


def _param_layout():
    """Deterministic packing of all [*,1] params into one [128, C] tensor."""
    cols = {}
    order = []
    for t, (di, do) in LINS.items():
        for m in range((do + 127) // 128):
            order.append((f"b_{t}", m, min(128, do - m * 128)))
    for m in range((2 * D) // 128):
        order.append(("b_d1s", m, 128))
    for t, d in LNS.items():
        for k in range(d // 128):
            order.append((f"g_{t}", k, 128))
            order.append((f"be_{t}", k, 128))
            if t in GELU_LNS:
                order.append((f"ge_{t}", k, 128))
                order.append((f"bee_{t}", k, 128))
    for j, (name, k, rows) in enumerate(order):
        cols[(name, k)] = (j, rows)
    return cols, len(order)


def _build(cmul):
    import concourse.mybir as mybir
    import concourse.tile as tile
    from concourse import bacc

    f32 = mybir.dt.float32
    AF = mybir.ActivationFunctionType
    OP = mybir.AluOpType
    AX = mybir.AxisListType

    nc = bacc.Bacc("TRN2", target_bir_lowering=False, debug=False)

    PCOLS, NPC = _param_layout()
    dr = {}
    for t, (di, do) in LINS.items():
        dr[f"wT_{t}"] = nc.dram_tensor(f"wT_{t}", [di, do], f32, kind="ExternalInput")
    dr["pmega"] = nc.dram_tensor("pmega", [128, NPC], f32, kind="ExternalInput")
    dr["fIT"] = nc.dram_tensor("fIT", [D, N], f32, kind="ExternalInput")
    dr["fTT"] = nc.dram_tensor("fTT", [D, N], f32, kind="ExternalInput")
    dr["cm2T"] = nc.dram_tensor("cm2T", [D, ND], f32, kind="ExternalInput")
    dr["centmix"] = nc.dram_tensor("centmix", [ND, D], f32, kind="ExternalInput")
    dr["mu2"] = nc.dram_tensor("mu2", [1, ND], f32, kind="ExternalInput")
    dr["selcol"] = nc.dram_tensor("selcol", [128, 16], f32, kind="ExternalInput")
    dr["selrow"] = nc.dram_tensor("selrow", [4, 512], f32, kind="ExternalInput")
    outT = nc.dram_tensor("outT", [4 * D + ND, N], f32, kind="ExternalOutput")

    with tile.TileContext(nc) as tc:
        from contextlib import ExitStack
        ctx = ExitStack()
        acts = ctx.enter_context(tc.tile_pool(name="acts", bufs=1))
        wpool = ctx.enter_context(tc.tile_pool(name="w", bufs=1))
        ppool = ctx.enter_context(tc.tile_pool(name="par", bufs=1))
        rows = ctx.enter_context(tc.tile_pool(name="rows", bufs=1))
        mmps = ctx.enter_context(tc.tile_pool(name="mmps", bufs=3, space="PSUM"))
        bcps = ctx.enter_context(tc.tile_pool(name="bcps", bufs=3, space="PSUM"))
        smps = ctx.enter_context(tc.tile_pool(name="smps", bufs=2, space="PSUM"))

        NSLOT = 64
        free_slots = list(range(NSLOT))
        _nm = [0]

        def nname(pfx):
            _nm[0] += 1
            return f"{pfx}{_nm[0]}"

        class Tl:
            __slots__ = ("slot", "t")

            def __init__(self):
                self.slot = free_slots.pop()
                self.t = acts.tile([128, N], f32, tag=f"a{self.slot}", name=nname("act"))

            def ap(self):
                return self.t[:]

            def free(self):
                if self.slot is not None:
                    free_slots.append(self.slot)
                    self.slot = None

        def anew():
            return Tl()

        def afree(ts):
            for t in ts:
                t.free()

        pmt = ppool.tile([128, NPC], f32, tag="pmt", bufs=1)
        nc.sync.dma_start(pmt[:], dr["pmega"].ap()[:, :])

        def ptile(name, k, rows_n=None):
            j, rows = PCOLS[(name, k)]
            return pmt[:rows, j:j + 1]

        def rtile(p, tag="r", bufs=5):
            return rows.tile([p, N], f32, tag=tag, bufs=bufs, name=nname("row"))

        ones_col = ppool.tile([128, 1], f32, tag="ones_col", bufs=1)
        nc.vector.memset(ones_col[:], 1.0)
        ones_r128 = rows.tile([1, 128], f32, tag="ones_r128", bufs=1)
        nc.vector.memset(ones_r128[:], 1.0)
        ones_row = rows.tile([1, N], f32, tag="ones_row", bufs=1)
        nc.vector.memset(ones_row[:], 1.0)
        ones14 = rows.tile([1, 4], f32, tag="ones14", bufs=1)
        nc.vector.memset(ones14[:], 1.0)
        selcol_t = ppool.tile([128, 16], f32, tag="selcol", bufs=1)
        nc.sync.dma_start(selcol_t[:], dr["selcol"].ap()[:, :])
        selrow_t = rows.tile([4, 512], f32, tag="selrow", bufs=1)
        nc.sync.dma_start(selrow_t[:], dr["selrow"].ap()[:, :])

        def linear(name, src, evict, chunk=8, after_m=None):
            """src: list of Tl k-tiles; evict(m, psum) consumes each psum."""
            di, do = LINS[name]
            kt, mt = di // 128, do // 128
            assert len(src) == kt and do % 128 == 0
            for m in range(mt):
                ps = mmps.tile([128, N], f32, tag="mm", name=nname("mmp"))
                k = 0
                for k0 in range(0, kt, chunk):
                    kb = min(chunk, kt - k0)
                    wt = wpool.tile([128, kb * 128], f32, tag="wsml", bufs=4, name=nname("w"))
                    src_ap = dr[f"wT_{name}"].ap().rearrange(
                        "(kt p) n -> p kt n", p=128)
                    nc.sync.dma_start(
                        wt[:].rearrange("p (kt n) -> p kt n", n=128),
                        src_ap[:, k0:k0 + kb, m * 128:(m + 1) * 128])
                    for kk in range(kb):
                        nc.tensor.matmul(ps[:], wt[:, kk * 128:(kk + 1) * 128],
                                         src[k].ap(), start=(k == 0),
                                         stop=(k == kt - 1))
                        k += 1
                evict(m, ps)
                if after_m and m in after_m:
                    after_m[m]()

        def evict_bias(name, out_list, func=None):
            AFunc = func or AF.Identity

            def ev(m, ps):
                t = anew()
                nc.scalar.activation(t.ap(), ps[:], AFunc,
                                     bias=ptile(f"b_{name}", m))
                out_list.append(t)
            return ev

        def ln_stats(tag, x):
            """Phase 1: PE stats matmuls + DVE row math. Returns state."""
            d = LNS[tag]
            kt = d // 128
            assert len(x) == kt
            st = smps.tile([33, N], f32, tag="stats", name=nname("st"))
            stA = st[0:1, :]    # x-sum, PE col group 0
            stB = st[32:33, :]  # x^2-sum, PE col group 1 (concurrent)
            for k in range(kt):
                sq = rtile(128, tag="sqr", bufs=2)
                nc.scalar.activation(sq[:], x[k].ap(), AF.Square)
                nc.tensor.matmul(stA, ones_col[:], x[k].ap(),
                                 start=(k == 0), stop=(k == kt - 1),
                                 skip_group_check=True)
                nc.tensor.matmul(stB, ones_col[:], sq[:],
                                 start=(k == 0), stop=(k == kt - 1),
                                 skip_group_check=True)
            mu = rtile(1)
            nc.vector.tensor_scalar(mu[:], stA, 1.0 / d, None, OP.mult)
            q = rtile(1)
            nc.vector.tensor_scalar(q[:], stB, 1.0 / d, None, OP.mult)
            u = rtile(1)
            nc.vector.scalar_tensor_tensor(u[:], mu[:], -1.0, mu[:], OP.mult, OP.mult)
            nc.vector.tensor_add(u[:], u[:], q[:])
            nc.vector.tensor_scalar(u[:], u[:], EPS, None, OP.add)
            r = rtile(1)
            nc.scalar.activation(r[:], u[:], AF.Sqrt)
            rc = rtile(1)
            nc.vector.reciprocal(rc[:], u[:])
            nc.vector.tensor_mul(r[:], r[:], rc[:])
            nc.vector.tensor_mul(rc[:], r[:], r[:])
            nc.vector.tensor_mul(rc[:], rc[:], u[:])
            nc.vector.tensor_scalar(rc[:], rc[:], -0.5, 1.5, OP.mult, OP.add)
            nc.vector.tensor_mul(r[:], r[:], rc[:])
            nc.vector.scalar_tensor_tensor(mu[:], mu[:], -1.0, r[:],
                                           OP.mult, OP.mult)
            return (tag, x, r, mu)

        def ln_apply(state, gelu=False):
            """Phase 2: broadcast matmuls + per-tile normalize (+erf-gelu)."""
            tag, x, r, mu = state
            kt = LNS[tag] // 128
            bs = bcps.tile([128, N], f32, tag="bc", name=nname("bc"))
            nc.tensor.matmul(bs[:], ones_r128[:], r[:], start=True, stop=True)
            bt = bcps.tile([128, N], f32, tag="bc", name=nname("bc"))
            nc.tensor.matmul(bt[:], ones_r128[:], mu[:], start=True, stop=True)
            sb = rtile(128, tag="bcs", bufs=2)
            nc.vector.tensor_copy(sb[:], bs[:])
            tb = rtile(128, tag="bcs", bufs=2)
            nc.vector.tensor_copy(tb[:], bt[:])
            out = []
            for k in range(kt):
                nrm = anew()
                nc.vector.tensor_mul(nrm.ap(), x[k].ap(), sb[:])
                nc.vector.tensor_add(nrm.ap(), nrm.ap(), tb[:])
                if not gelu:
                    t = anew()
                    nc.scalar.activation(t.ap(), nrm.ap(), AF.Identity,
                                         bias=ptile(f"be_{tag}", k),
                                         scale=ptile(f"g_{tag}", k))
                    out.append(t)
                    nrm.free()
                else:
                    xln = anew()
                    nc.scalar.activation(xln.ap(), nrm.ap(), AF.Identity,
                                         bias=ptile(f"be_{tag}", k),
                                         scale=ptile(f"g_{tag}", k))
                    er = anew()
                    nc.scalar.activation(er.ap(), nrm.ap(), AF.Erf,
                                         bias=ptile(f"bee_{tag}", k),
                                         scale=ptile(f"ge_{tag}", k))
                    t = anew()
                    nc.vector.scalar_tensor_tensor(t.ap(), er.ap(), 1.0, xln.ap(),
                                                   OP.add, OP.mult)
                    out.append(t)
                    xln.free(); er.free(); nrm.free()
            return out

        def layer_norm(tag, x, gelu=False):
            return ln_apply(ln_stats(tag, x), gelu=gelu)

        # ================= forward =================
        fI, fT = [], []
        for k in range(8):
            t = anew(); nc.gpsimd.dma_start(t.ap(), dr["fIT"].ap()[k * 128:(k + 1) * 128, :]); fI.append(t)
            t = anew(); nc.scalar.dma_start(t.ap(), dr["fTT"].ap()[k * 128:(k + 1) * 128, :]); fT.append(t)

        def sa_l2(br, fin, hg, after_m=None):
            out = []

            def ev(m, ps):
                t = anew()
                nc.scalar.activation(t.ap(), ps[:], AF.Identity,
                                     bias=ptile(f"b_sa{br}_l2", m))
                nc.vector.tensor_add(t.ap(), t.ap(), fin[m].ap())
                out.append(t)
            linear(f"sa{br}_l2", hg, ev, after_m=after_m)
            return out

        # interleaved; LN applies embedded early in the next linear's m-loop
        h1I = []
        linear("saI_l1", fI, evict_bias("saI_l1", h1I), chunk=2)
        stI = ln_stats("saI", h1I)
        h1T = []
        hgI, hgT = [], []

        def _apI():
            hgI.extend(ln_apply(stI, gelu=True)); afree(h1I)
        linear("saT_l1", fT, evict_bias("saT_l1", h1T), after_m={0: _apI})
        stT = ln_stats("saT", h1T)

        def _apT():
            hgT.extend(ln_apply(stT, gelu=True)); afree(h1T)
        fIr = sa_l2("I", fI, hgI, after_m={0: _apT})
        afree(hgI)
        fTr = sa_l2("T", fT, hgT)
        afree(hgT)
        afree(fI); afree(fT)

        def gca_pre(cname, kv, res):
            # o_proj(v_proj(kv)) composed into one matmul on the host
            pre = []

            def ev(m, ps):
                t = anew()
                nc.scalar.activation(t.ap(), ps[:], AF.Identity,
                                     bias=ptile(f"b_{cname}", m))
                nc.vector.tensor_add(t.ap(), t.ap(), res[m].ap())
                pre.append(t)
            linear(cname, kv, ev)
            return pre

        preI = gca_pre("g_cI", fTr, fIr)
        stLI = ln_stats("lnI", preI)
        preT = gca_pre("g_cT", fIr, fTr)
        stLT = ln_stats("lnT", preT)
        afree(fIr); afree(fTr)
        oI = ln_apply(stLI)
        afree(preI)
        # token-0 q1 projection (only needs oI); lnT apply embedded after m1
        q1_t0, v0 = [], []
        oT = []

        def _apLT():
            oT.extend(ln_apply(stLT)); afree(preT)
        linear("q1", oI, evict_bias("q1", q1_t0), after_m={1: _apLT})
        fused = oI + oT  # 16 k-tiles

        # lambda path (small) — interleaved into attention below
        lam_h = []
        linear("lam1", fused, evict_bias("lam1", lam_h, func=AF.Relu))
        wl2 = wpool.tile([128, 2], f32, tag="wtiny", bufs=2, name=nname("w"))
        nc.sync.dma_start(
            wl2[:].rearrange("p (kt n) -> p kt n", n=1),
            dr["wT_lam2"].ap().rearrange("(kt p) n -> p kt n", p=128)[:, :, :])
        lam_ps = smps.tile([1, N], f32, tag="stats", name=nname("st"))
        for k in range(2):
            nc.tensor.matmul(lam_ps[:], wl2[:, k:k + 1], lam_h[k].ap(),
                             start=(k == 0), stop=(k == 1))
        afree(lam_h)
        lam = rtile(1, tag="lam", bufs=1)
        nc.scalar.activation(lam[:], lam_ps[:], AF.Sigmoid,
                             bias=ptile("b_lam2", 0, 1))
        lam4_ps = smps.tile([4, N], f32, tag="stats", name=nname("st"))
        nc.tensor.matmul(lam4_ps[:], ones14[:], lam[:], start=True, stop=True)
        lam4 = rtile(4, tag="lam4", bufs=1)
        nc.vector.tensor_copy(lam4[:], lam4_ps[:])
        om4 = rtile(4, tag="om4", bufs=1)
        nc.vector.tensor_scalar(om4[:], lam4[:], -1.0, 1.0, OP.mult, OP.add)

        # K-diff projections: K0-K1 = W_k @ (tok0 - tok1)
        tdf = []
        for k in range(8):
            t = anew()
            nc.vector.tensor_sub(t.ap(), oI[k].ap(), oT[k].ap())
            tdf.append(t)

        def ev_copy(lst):
            def ev(m, ps):
                t = anew()
                nc.scalar.copy(t.ap(), ps[:])
                lst.append(t)
            return ev
        kd1, kd2 = [], []
        linear("k1", tdf, ev_copy(kd1))
        linear("k2", tdf, ev_copy(kd2))
        afree(tdf)

        SCALE = float(HD ** -0.5)

        def score_rows2(q1t, q2t):
            """Both branch scores packed into PE col groups 0 and 1."""
            sc = smps.tile([36, N], f32, tag="stats", name=nname("st"))
            for k in range(8):
                p1 = rtile(128, tag="prod", bufs=4)
                nc.vector.tensor_mul(p1[:], q1t[k].ap(), kd1[k].ap())
                nc.tensor.matmul(sc[0:4, :], selcol_t[:, 4 * (k // 2):4 * (k // 2) + 4],
                                 p1[:], start=(k == 0), stop=(k == 7),
                                 skip_group_check=True)
                p2 = rtile(128, tag="prod", bufs=4)
                nc.vector.tensor_mul(p2[:], q2t[k].ap(), kd2[k].ap())
                nc.tensor.matmul(sc[32:36, :], selcol_t[:, 4 * (k // 2):4 * (k // 2) + 4],
                                 p2[:], start=(k == 0), stop=(k == 7),
                                 skip_group_check=True)
            a1 = rtile(4, tag="arow", bufs=3)
            nc.scalar.activation(a1[:], sc[0:4, :], AF.Sigmoid, scale=SCALE)
            a2 = rtile(4, tag="arow", bufs=3)
            nc.scalar.activation(a2[:], sc[32:36, :], AF.Sigmoid, scale=SCALE)
            return a1, a2

        def attn_w(a1, a2, ti):
            u = rtile(4, tag="arow", bufs=3)
            nc.vector.tensor_mul(u[:], lam4[:], a2[:])
            nc.vector.tensor_sub(u[:], a1[:], u[:])
            w0 = rtile(4, tag=f"w{ti}0", bufs=1)
            nc.vector.tensor_scalar(w0[:], u[:], 0.0, None, OP.max)
            w1 = rtile(4, tag=f"w{ti}1", bufs=1)
            nc.vector.tensor_sub(w1[:], om4[:], u[:])
            nc.vector.tensor_scalar(w1[:], w1[:], 0.0, None, OP.max)
            return w0, w1

        # token 0 (q1_t0 computed above, during lnT apply window)
        q2_t0 = []
        linear("q2", oI, evict_bias("q2", q2_t0))
        a1_t0, a2_t0 = score_rows2(q1_t0, q2_t0)
        afree(q1_t0); afree(q2_t0)
        wq0 = attn_w(a1_t0, a2_t0, 0)
        linear("v", oI, evict_bias("v", v0))
        # token 1
        q1_t1 = []
        linear("q1", oT, evict_bias("q1", q1_t1))
        q2_t1 = []
        linear("q2", oT, evict_bias("q2", q2_t1))
        a1_t1, a2_t1 = score_rows2(q1_t1, q2_t1)
        afree(q1_t1); afree(q2_t1)
        wq1 = attn_w(a1_t1, a2_t1, 1)
        afree(kd1); afree(kd2)
        v1 = []
        linear("v", oT, evict_bias("v", v1))

        x = []

        def combine_oproj(ti, w01, res):
            w0, w1 = w01
            att = []
            for h in range(4):
                b0 = bcps.tile([128, N], f32, tag="bc", name=nname("bc"))
                nc.tensor.matmul(b0[:], selrow_t[:, 128 * h:128 * (h + 1)], w0[:],
                                 start=True, stop=True)
                b1 = bcps.tile([128, N], f32, tag="bc", name=nname("bc"))
                nc.tensor.matmul(b1[:], selrow_t[:, 128 * h:128 * (h + 1)], w1[:],
                                 start=True, stop=True)
                for k in (2 * h, 2 * h + 1):
                    r1 = rtile(128, tag="prod", bufs=4)
                    nc.vector.tensor_mul(r1[:], v0[k].ap(), b0[:])
                    t = anew()
                    nc.vector.tensor_mul(t.ap(), v1[k].ap(), b1[:])
                    nc.vector.tensor_add(t.ap(), t.ap(), r1[:])
                    att.append(t)

            def ev(m, ps, _res=res):
                t = anew()
                nc.scalar.activation(t.ap(), ps[:], AF.Identity,
                                     bias=ptile("b_o", m))
                nc.vector.tensor_add(t.ap(), t.ap(), _res[m].ap())
                x.append(t)
            linear("o", att, ev)
            afree(att)

        combine_oproj(0, wq0, oI)
        afree(oI)
        combine_oproj(1, wq1, oT)
        afree(v0); afree(v1)
        afree(oT)

        # ---- c/s paths interleaved, cdist launched early ----
        h1c = []
        linear("c1", x, evict_bias("c1", h1c))
        st_c1 = ln_stats("cln1", h1c)
        h1s = []
        hgc, hgs, c = [], [], []

        def _apC1():
            hgc.extend(ln_apply(st_c1, gelu=True)); afree(h1c)
        linear("s1", x, evict_bias("s1", h1s), after_m={1: _apC1})
        st_s1 = ln_stats("sln1", h1s)
        afree(x)
        h2c = []

        def _apS1():
            hgs.extend(ln_apply(st_s1, gelu=True)); afree(h1s)
        linear("c2", hgc, evict_bias("c2", h2c), after_m={1: _apS1})
        afree(hgc)
        st_c2 = ln_stats("cln2", h2c)
        h2s = []

        def _apC2():
            c.extend(ln_apply(st_c2)); afree(h2c)
        linear("s2", hgs, evict_bias("s2", h2s), after_m={1: _apC2})
        afree(hgs)
        st_s2 = ln_stats("sln2", h2s)

        # cdist part 1: dt2 = ||mu||^2 - 2 mu.c -> one-hot (gpsimd latency
        # covered by the s apply + d1/d2 matmuls below)
        d2ps = smps.tile([8, N], f32, tag="stats", name=nname("st"))
        wcm = wpool.tile([128, 8 * ND], f32, tag="wtiny", bufs=2, name=nname("w"))
        nc.sync.dma_start(
            wcm[:].rearrange("p (kt n) -> p kt n", n=ND),
            dr["cm2T"].ap().rearrange("(kt p) n -> p kt n", p=128)[:, :, :])
        for k in range(8):
            nc.tensor.matmul(d2ps[:ND, :], wcm[:, k * ND:(k + 1) * ND], c[k].ap(),
                             start=(k == 0), stop=False)
        mu2t = rows.tile([1, ND], f32, tag="mu2", bufs=1, name=nname("row"))
        nc.sync.dma_start(mu2t[:], dr["mu2"].ap()[:, :])
        nc.tensor.matmul(d2ps[:ND, :], mu2t[:], ones_row[:], start=False, stop=True)
        d2s = rtile(ND, tag="d2s", bufs=1)
        nc.vector.tensor_copy(d2s[:], d2ps[:ND, :])
        mx7 = rtile(ND, tag="mx7", bufs=1)
        import concourse.bass_isa as bass_isa
        nc.gpsimd.partition_all_reduce(mx7[:], d2s[:], channels=ND,
                                       reduce_op=bass_isa.ReduceOp.max)
        oneh = rtile(ND, tag="oneh", bufs=1)
        nc.vector.tensor_tensor(oneh[:], d2s[:], mx7[:], OP.is_equal)

        s = ln_apply(st_s2)
        afree(h2s)

        # domain classifier stage 1 on s (m2/m3 issued later, between
        # dense d1/hardest streams, so PE never idles on the thin chain)
        dm1 = []
        linear("m1", s, evict_bias("m1", dm1, func=AF.Relu))
        wm2 = wpool.tile([128, 64], f32, tag="wtiny", bufs=2, name=nname("w"))
        nc.sync.dma_start(wm2[:], dr["wT_m2"].ap()[:, :])
        wm3 = wpool.tile([128, ND], f32, tag="wtiny", bufs=2, name=nname("w"))
        nc.sync.dma_start(wm3[:64, :], dr["wT_m3"].ap()[:, :])

        # h_recon = d2(gelu(d1([c, s]))), 0.5 folded into wT_d2
        cat = c + s
        hg = []

        def ev_d1(m, ps):
            xt = anew()
            nc.scalar.activation(xt.ap(), ps[:], AF.Identity,
                                 bias=ptile("b_d1", m))
            er = anew()
            nc.scalar.activation(er.ap(), ps[:], AF.Erf,
                                 bias=ptile("b_d1s", m), scale=ISQ2)
            t = anew()
            nc.vector.scalar_tensor_tensor(t.ap(), er.ap(), 1.0, xt.ap(),
                                           OP.add, OP.mult)
            hg.append(t)
            xt.free(); er.free()
        linear("d1", cat, ev_d1)

        ps2 = mmps.tile([128, N], f32, tag="mm", name=nname("mmp"))
        nc.tensor.matmul(ps2[:64, :], wm2[:, :64], dm1[0].ap(),
                         start=True, stop=True)
        afree(dm1)
        dm2 = anew()
        nc.scalar.activation(dm2.ap()[:64, :], ps2[:64, :], AF.Relu,
                             bias=ptile("b_m2", 0))

        # c_do = cmul*c + centmix.T @ onehot (before d2 so c frees early
        # and the thin hardest matmuls hide inside the dense stream)
        cmixt = wpool.tile([ND, D], f32, tag="cmix", bufs=1, name=nname("w"))
        nc.sync.dma_start(cmixt[:], dr["centmix"].ap()[:, :])
        for m in range(8):
            hp = mmps.tile([128, N], f32, tag="mm", name=nname("mmp"))
            nc.tensor.matmul(hp[:], cmixt[:, m * 128:(m + 1) * 128], oneh[:],
                             start=True, stop=True)
            cd = anew()
            nc.vector.scalar_tensor_tensor(cd.ap(), c[m].ap(), cmul, hp[:],
                                           OP.mult, OP.add)
            nc.sync.dma_start(outT.ap()[m * 128:(m + 1) * 128, :], cd.ap())
            cd.free()
        afree(c)

        ps3 = mmps.tile([128, N], f32, tag="mm", name=nname("mmp"))
        nc.tensor.matmul(ps3[:ND, :], wm3[:64, :ND], dm2.ap()[:64, :],
                         start=True, stop=True)
        dm2.free()
        dm3 = anew()
        nc.scalar.activation(dm3.ap()[:ND, :], ps3[:ND, :], AF.Identity,
                             bias=ptile("b_m3", 0))
        nc.sync.dma_start(outT.ap()[4 * D:4 * D + ND, :], dm3.ap()[:ND, :])
        dm3.free()

        def ev_d2(m, ps):
            t = anew()
            nc.scalar.activation(t.ap(), ps[:], AF.Identity,
                                 bias=ptile("b_d2", m))
            nc.sync.dma_start(
                outT.ap()[2 * D + m * 128:2 * D + (m + 1) * 128, :], t.ap())
            t.free()
        linear("d2", hg, ev_d2)
        afree(hg)

        for m, t in enumerate(s):
            nc.sync.dma_start(outT.ap()[D + m * 128:D + (m + 1) * 128, :], t.ap())
            t.free()

        ctx.close()

    nc.compile()
    return nc


def _prepare_inputs(f_I, f_T, params, centroids, counts):
    p = params
    W = {}

    def put(tag, lin, half=False):
        w = _f32(lin["w"])
        b = _f32(lin["b"])
        wT = np.ascontiguousarray(w.T)
        if half:
            wT = np.ascontiguousarray(np.float32(0.5) * wT)
        W[f"wT_{tag}"] = wT
        W[f"b_{tag}"] = np.ascontiguousarray(b.reshape(-1, 1))

    put("saI_l1", p["sa_I"]["l1"]); put("saI_l2", p["sa_I"]["l2"], half=True)
    put("saT_l1", p["sa_T"]["l1"]); put("saT_l2", p["sa_T"]["l2"], half=True)
    for br in ("I", "T"):
        wv = _f32(p["gca"][f"v_{br}"]["w"]).astype(np.float64)
        bv = _f32(p["gca"][f"v_{br}"]["b"]).astype(np.float64)
        wo = _f32(p["gca"][f"o_{br}"]["w"]).astype(np.float64)
        bo = _f32(p["gca"][f"o_{br}"]["b"]).astype(np.float64)
        W[f"wT_g_c{br}"] = np.ascontiguousarray((wo @ wv).astype(np.float32).T)
        W[f"b_g_c{br}"] = np.ascontiguousarray(
            (wo @ bv + bo).astype(np.float32).reshape(-1, 1))
    for t in ("q1", "q2", "k1", "k2", "v", "o", "lam1", "lam2"):
        put(t, p["diff"][t])
    put("c1", p["dis"]["c1"]); put("c2", p["dis"]["c2"], half=True)
    put("s1", p["dis"]["s1"]); put("s2", p["dis"]["s2"], half=True)
    put("d1", p["dis"]["d1"]); put("d2", p["dis"]["d2"], half=True)
    put("m1", p["dom"]["l1"]); put("m2", p["dom"]["l2"]); put("m3", p["dom"]["l3"])

    LNMAP = {"saI": p["sa_I"]["ln"], "saT": p["sa_T"]["ln"],
             "lnI": p["gca"]["ln_I"], "lnT": p["gca"]["ln_T"],
             "cln1": p["dis"]["cln1"], "cln2": p["dis"]["cln2"],
             "sln1": p["dis"]["sln1"], "sln2": p["dis"]["sln2"]}
    ISQ = np.float32(ISQ2)
    for t, ln in LNMAP.items():
        g = _f32(ln["g"]).reshape(-1, 1)
        b = _f32(ln["b"]).reshape(-1, 1)
        W[f"g_{t}"] = np.ascontiguousarray(g)
        W[f"be_{t}"] = np.ascontiguousarray(b)
        if t in GELU_LNS:
            W[f"ge_{t}"] = np.ascontiguousarray(g * ISQ)
            W[f"bee_{t}"] = np.ascontiguousarray(b * ISQ)
    W["b_d1s"] = np.ascontiguousarray(W["b_d1"] * ISQ)

    cent = _f32(centroids)
    counts_sum = float(np.asarray(counts, np.float64).sum())
    fmix = MIX if counts_sum > 0 else 0.0
    W["cm2T"] = np.ascontiguousarray((np.float32(-2.0) * cent).T)
    W["centmix"] = np.ascontiguousarray(np.float32(fmix) * cent)
    try:
        import jax
        cpu = jax.devices("cpu")[0]
        with jax.default_device(cpu):
            import jax.numpy as jnp
            mu2 = np.asarray((jnp.asarray(cent) ** 2).sum(-1), np.float32)
    except Exception:
        mu2 = (cent.astype(np.float64) ** 2).sum(-1).astype(np.float32)
    W["mu2"] = np.ascontiguousarray(mu2.reshape(1, ND))

    selcol = np.zeros((128, 16), np.float32)
    selrow = np.zeros((4, 512), np.float32)
    for h in range(4):
        selcol[:, 4 * h + h] = 1.0
        selrow[h, 128 * h:128 * (h + 1)] = 1.0
    W["selcol"] = selcol
    W["selrow"] = selrow

    PCOLS, NPC = _param_layout()
    pmega = np.zeros((128, NPC), np.float32)
    for (name, k), (j, rows) in PCOLS.items():
        pmega[:rows, j] = W[name][k * 128:k * 128 + rows, 0]
    W["pmega"] = pmega

    fI = _f32(f_I)
    fT = _f32(f_T)
    in_maps = []
    for cix in range(NCORES):
        m = dict(W)
        m["fIT"] = np.ascontiguousarray(fI[cix * N:(cix + 1) * N, :].T)
        m["fTT"] = np.ascontiguousarray(fT[cix * N:(cix + 1) * N, :].T)
        in_maps.append(m)
    return in_maps, float(1.0 - fmix)


def kernel(f_I, f_T, params, centroids, counts):
    from concourse import bass_utils

    in_maps, cmul = _prepare_inputs(f_I, f_T, params, centroids, counts)
    key = ("prog", cmul)
    if key not in _CACHE:
        _CACHE[key] = _build(cmul)
    nc = _CACHE[key]

    trace = os.environ.get("KERNEL_TRACE") == "1"
    if trace:
        try:
            import sys, types
            if "antenv.axon_hooks" not in sys.modules:
                import antenv  # noqa: F401
                from trn_agent_boot.trn_boot import _ntff_profile_via_ctypes
                hook = _ntff_profile_via_ctypes("/opt/axon/libaxon_pjrt.so")
                mod = types.ModuleType("antenv.axon_hooks")
                mod.get_axon_ntff_profile_hook = lambda: hook
                mod.set_axon_ntff_profile_hook = lambda h: None
                sys.modules["antenv.axon_hooks"] = mod
        except Exception as e:
            print("trace hook install failed:", e)
            trace = False

    res = bass_utils.run_bass_kernel_spmd(
        nc, in_maps, core_ids=list(range(NCORES)), trace=trace)
    global LAST_EXEC_NS
    if trace and res.exec_time_ns is not None:
        LAST_EXEC_NS = int(res.exec_time_ns)
        print(f"HW exec time: {res.exec_time_ns} ns")

    out = np.empty((B, 4 * D + ND), np.float32)
    for cix in range(NCORES):
        out[cix * N:(cix + 1) * N, :] = res.results[cix]["outT"].T
    return out
